# revision 8
# baseline (speedup 1.0000x reference)
"""Trainium2 Bass kernel for nn_Attention_44830868635854.

Fused: 1x1-conv QKV -> depthwise 3x3 on q -> 8-head attention (softmax) ->
ReLU -> 1x1 proj -> GroupNorm(8).

Sharding: 8 cores = (batch b in 0..3) x (spatial half s in 0..1). Each core
computes output rows [24s, 24s+24) of the 48x48 image for its batch (1152
query pixels) across all 8 heads, using the full image for k/v. GroupNorm
statistics are combined across the core pair with a tiny AllReduce.

Main-loop structure (per core), NT=128 query tile, MT=128 key tile:
  unit u=(j,i): 8 logit matmuls (bf16 q stream vs f32r k weights) into a
  rotating 2-bank PSUM slot (3 slots); exp of the [128,1024] logit block on
  either the ACT engine (native Exp -> bf16) or the DVE engine (custom
  single-instruction quartic-polynomial exp, ~1e-3 rel err) -- the exp
  elementwise work is the kernel's roofline so it is split across both
  engines; AV uses P as the PE *weights* (lhsT) with a tiny [128,17] v/ones
  rhs, accumulating O^T and the softmax denominator S in PSUM across all 18
  key tiles. Finalize per j: reciprocal of S, fused relu+normalize
  (scalar_tensor_tensor), PE transpose back to channel-major, 1x1 proj, and
  GroupNorm partial sums.
"""

from collections import deque

import numpy as np
import ml_dtypes

import concourse.bass as bass
import concourse.mybir as mybir
import concourse.tile as tile
from concourse.tile import add_dep_helper
from concourse.bass_utils import run_bass_kernel_spmd
from concourse import dve_ops as _DO
from concourse.dve_spec import Spec, Src0, Src1, C0, C1, C2, One, lower as _dve_lower
from concourse.dve_uop import DveOpSpec as _DveOpSpec

F32 = mybir.dt.float32
F32R = mybir.dt.float32r
BF16 = mybir.dt.bfloat16
AF = mybir.ActivationFunctionType
ALU = mybir.AluOpType
NPBF16 = ml_dtypes.bfloat16

B, DIM, H, W = 4, 128, 48, 48
HEADS, HEAD_DIM = 8, 16
N = H * W            # 2304
ROWS_HALF = 24
NSL = ROWS_HALF * W  # 1152 per core
NT = 128             # query tile (9 per core)
MT = 128             # key tile (18 per core)
NJ = NSL // NT       # 9
NI = N // MT         # 18
EPS = 1e-5
GN_DIV = 1.0 / (16.0 * N)

# quartic exp(L/4) fit over L in [-4.4, 4.3]: max rel err 9.5e-4
# P(L) = (((L*K4 + K3)*L + K2)*L + K1)*L + 1
K4 = 0.00015327319036728373
K3 = 0.002763773359872127
K2 = 0.03147120315761681
K1 = 0.24957119869968478

# exp engine split: DVE for these u%7 slots (3/7 ~ 69 of 162 units)
EXP_DVE_MOD = 7
EXP_DVE_PAT = (2, 5, 6)


def _register_exp4():
    """Register a custom single-instruction DVE op computing the quartic
    exp approximation (Horner, P(0)=1 exact). Data-driven: opcode row +
    per-NEFF uop table, same mechanism as the stock custom DVE ops."""
    name = "EXP_POLY4_ANT"
    if name in _DO._SUB_OPCODE_FOR_NAME:
        for op in _DO.OPS:
            if op.name == name:
                return op
    body = (((Src0 * C0 + C1) * Src0 + C2) * Src0 + Src1) * Src0 + One

    def ref(in0, in1, c0, c1, c2):
        x = in0.astype(np.float32)
        return ((((x * c0 + c1) * x + c2) * x + in1) * x + 1.0).astype(np.float32)

    spec = Spec(body=body, reference=ref)
    row = _DO._CUSTOM_DVE_ROW_BASE + len(_DO.OPS)
    assert row < 0x20
    shas = {}
    for ver in ("v3", "v4"):
        s = _DveOpSpec(name=name, opcode=row, uops=_dve_lower(spec, ver=ver),
                       rd1_en=True)
        shas[ver] = s.sha(ver)
    op = _DO.DveOp(name, spec, subdim=False, uops_sha=shas)
    _DO.OPS.append(op)
    _DO.CUSTOM_DVE_SPECS[name] = spec
    _DO._SUB_OPCODE_FOR_NAME[name] = row
    return op


EXP4 = _register_exp4()


def _split_multi_waits(nc):
    """walrus here allows one sync-wait slot per lowered instruction; move
    extra waits onto standalone EventSemaphore instructions."""
    for func in nc.m.functions:
        for block in func.blocks:
            new_insts = []
            for inst in block.instructions:
                si = inst.sync_info
                waits = list(si.on_wait) if si is not None and si.on_wait else []
                if len(waits) > 1 and not isinstance(inst, mybir.InstEventSemaphore):
                    for k, w in enumerate(waits[:-1]):
                        new_insts.append(
                            mybir.InstEventSemaphore(
                                name=f"{inst.name}_wsplit{k}",
                                engine=inst.engine,
                                ins=[],
                                outs=[],
                                sync_info=mybir.SyncInfo(on_wait=[w], on_update=[]),
                            )
                        )
                    si.on_wait = waits[-1:]
                new_insts.append(inst)
            block.instructions[:] = new_insts


def _build(with_cc=True):
    nc = bass.Bass()
    dt = nc.dram_tensor

    xb_d = dt("xb", [DIM, N], BF16, kind="ExternalInput")
    xq_d = dt("xq", [DIM, 26 * 50], BF16, kind="ExternalInput")
    wk_d = dt("wk", [DIM, 2, 128], BF16, kind="ExternalInput")
    w2_d = dt("w2", [DIM, 18, 128], BF16, kind="ExternalInput")
    bq_d = dt("bq", [128, 2], F32, kind="ExternalInput")
    wv_d = dt("wv", [DIM, 136], BF16, kind="ExternalInput")
    bvr_d = dt("bvr", [128, 136], BF16, kind="ExternalInput")
    wpj_d = dt("wpj", [68, 2, 128], BF16, kind="ExternalInput")
    idn_d = dt("idn", [DIM, 128], BF16, kind="ExternalInput")
    gab_d = dt("gab", [DIM, 2], F32, kind="ExternalInput")  # gn gamma | beta
    gsel_d = dt("gsel", [DIM, 8], F32, kind="ExternalInput")

    out_d = dt("out_half", [DIM, NSL], F32, kind="ExternalOutput")

    cc_in = dt("cc_in", [8, 2], F32)
    cc_out = dt("cc_out", [8, 2], F32)
    scratch_d = dt("scratch", [128, 1], F32)

    with tile.TileContext(nc) as tc:
        with (
            tc.tile_pool(name="persist", bufs=1) as pp,
            tc.tile_pool(name="ptp", bufs=3) as ptp,
            tc.tile_pool(name="fin", bufs=2) as finp,
            tc.tile_pool(name="lp", bufs=1, space="PSUM") as lpp,
        ):
            lpbig = lpp.tile([128, 8, 512], F32, tag="lpbig")
            lpb16 = lpbig.bitcast(BF16)  # [128, 8, 1024]
            psum_rr = [0]

            def psum_bank():
                b = psum_rr[0] % 8
                psum_rr[0] += 1
                return lpbig[:, b : b + 1, :]

            # drain engine alternation: weighted ACT/DVE copy
            drain_rr = [0]

            def drain_copy(out, in_):
                drain_rr[0] += 1
                if drain_rr[0] % 2 == 0:
                    nc.scalar.copy(out=out, in_=in_)
                else:
                    nc.vector.tensor_copy(out=out, in_=in_)

            # ---- ACT exp table preload (single-wait discipline for hot loop)
            dummy = pp.tile([128, 1], F32, tag="dummy")
            nc.vector.memset(dummy, 0.0)
            nc.scalar.activation(out=dummy, in_=dummy, func=AF.Exp)
            nc.gpsimd.dma_start(out=scratch_d[:, :], in_=dummy)

            # exp4 k1 constant (per-partition broadcast source)
            k1t = pp.tile([128, 1], F32, tag="k1t")
            nc.gpsimd.memset(k1t, K1)

            # ---- load inputs (pre-formatted on host; no on-device conversion)
            xb = pp.tile([DIM, N], BF16, tag="xb")
            nc.sync.dma_start(out=xb, in_=xb_d[:, :])
            xq = pp.tile([DIM, 26 * 50], BF16, tag="xq")
            nc.sync.dma_start(out=xq, in_=xq_d[:, :])
            wkt = pp.tile([DIM, 2, 128], BF16, tag="wkt")
            nc.sync.dma_start(out=wkt, in_=wk_d[:, :, :])
            wk = [wkt[:, g, :] for g in range(2)]
            w2t = pp.tile([DIM, 18, 128], BF16, tag="w2t")
            nc.sync.dma_start(out=w2t, in_=w2_d[:, :, :])
            w2 = [w2t[:, i, :] for i in range(18)]
            bqt = pp.tile([128, 2], F32, tag="bqt")
            nc.sync.dma_start(out=bqt, in_=bq_d[:, :])
            bqv = [bqt[:, g : g + 1] for g in range(2)]
            wv = pp.tile([DIM, 136], BF16, tag="wv")
            nc.sync.dma_start(out=wv, in_=wv_d[:, :])
            bvr = pp.tile([128, 136], BF16, tag="bvr")
            nc.sync.dma_start(out=bvr, in_=bvr_d[:, :])
            wpj = pp.tile([68, 2, 128], BF16, tag="wpj")
            nc.sync.dma_start(out=wpj, in_=wpj_d[:, :, :])
            idn = pp.tile([DIM, 128], BF16, tag="idn")
            nc.sync.dma_start(out=idn, in_=idn_d[:, :])
            gab = pp.tile([DIM, 2], F32, tag="gab")
            nc.sync.dma_start(out=gab, in_=gab_d[:, :])
            gsel = pp.tile([DIM, 8], F32, tag="gsel")
            nc.sync.dma_start(out=gsel, in_=gsel_d[:, :])

            # ---- QKV projections ----
            kg = [None, None]
            qg = [None, None]

            def emit_k(g):
                kt = pp.tile([DIM, N], F32R, tag=f"kg{g}", name=f"kg{g}")
                for j0 in range(0, N, 512):
                    n = min(512, N - j0)
                    ps = psum_bank()
                    nc.tensor.matmul(
                        out=ps[:, 0, 0:n], lhsT=wk[g], rhs=xb[:, j0 : j0 + n],
                        start=True, stop=True,
                    )
                    drain_copy(kt[:, j0 : j0 + n], ps[:, 0, 0:n])
                kg[g] = kt

            def emit_q(g):
                qt = pp.tile([128, NSL], BF16, tag=f"qg{g}", name=f"qg{g}")
                xqv = xq.rearrange("p (r c) -> p r c", c=50)
                for blk in range(3):  # 8 output rows each
                    ps = psum_bank()
                    for ty in range(3):
                        for tx in range(3):
                            tap = 3 * ty + tx
                            nc.tensor.matmul(
                                out=ps[:, 0, 0:384],
                                lhsT=w2[9 * g + tap],
                                rhs=xqv[:, 8 * blk + ty : 8 * blk + ty + 8,
                                        tx : tx + W],
                                start=(tap == 0), stop=(tap == 8),
                            )
                    nc.vector.tensor_scalar_add(
                        out=qt[:, blk * 384 : (blk + 1) * 384],
                        in0=ps[:, 0, 0:384],
                        scalar1=bqv[g],
                    )
                qg[g] = qt

            vt = pp.tile([128, NI, 136], BF16, tag="vt")

            def emit_vt(i):
                # bank 7 halves; free during the first j-tile's units
                col = 256 * (i % 2)
                nc.tensor.matmul(
                    out=lpbig[:, 7, col : col + 136],
                    lhsT=xb[:, i * MT : (i + 1) * MT],
                    rhs=wv, start=True, stop=True,
                )
                if i % 2 == 1:
                    pair = lpbig[:, 7, :].rearrange(
                        "p (a b) -> p a b", b=256)[:, :, 0:136]
                    nc.vector.tensor_add(
                        out=vt[:, i - 1 : i + 1, :], in0=pair,
                        in1=bvr.rearrange("p (a b) -> p a b", a=1
                                          ).broadcast_to([128, 2, 136]),
                    )

            emit_k(0)
            emit_k(1)
            emit_q(0)
            emit_q(1)

            # ---- attention main loop ----
            o2 = pp.tile([DIM, NSL], BF16, tag="o2")
            sqd = pp.tile([DIM, NSL], BF16, tag="sqd")
            s1p = pp.tile([DIM, NJ], F32, tag="s1p")
            s2p = pp.tile([DIM, NJ], F32, tag="s2p")
            units = [(j, i) for j in range(NJ) for i in range(NI)]

            def emit_logits(u):
                j, i = units[u]
                z = u % 3
                js = slice(j * NT, (j + 1) * NT)
                for g in range(2):
                    for jj in range(4):
                        h = 4 * g + jj
                        nc.tensor.matmul(
                            out=lpbig[:, 2 * z + h // 4,
                                      128 * (h % 4) : 128 * (h % 4) + 128],
                            lhsT=kg[g][32 * jj : 32 * jj + 16,
                                       i * MT : (i + 1) * MT],
                            rhs=qg[g][32 * jj : 32 * jj + 16, js],
                            start=True, stop=True,
                            tile_position=(32 * jj, 0),
                        )

            def emit_exp(u):
                z = u % 3
                src = lpbig[:, 2 * z : 2 * z + 2, :].rearrange("p a b -> p (a b)")
                pt = ptp.tile([128, 8, 128], BF16, tag="pt", name="pt")
                flat = pt.rearrange("p a b -> p (a b)")
                if (u % EXP_DVE_MOD) in EXP_DVE_PAT:
                    nc.vector._custom_dve(
                        EXP4, out=flat, in0=src,
                        in1=k1t[:, 0:1].broadcast_to([128, 1024]),
                        s0=K4, s1=K3, imm2=K2,
                    )
                else:
                    nc.scalar.activation(out=flat, in_=src, func=AF.Exp, scale=0.25)
                return pt

            def emit_av(u, pt):
                j, i = units[u]
                half = 256 * (j % 2)  # double-buffered AV accumulator
                for h in range(HEADS):
                    nc.tensor.matmul(
                        out=lpbig[:, 6, half + 32 * h : half + 32 * h + 17],
                        lhsT=pt[:, h, :],
                        rhs=vt[:, i, 17 * h : 17 * h + 17],
                        start=(i == 0), stop=(i == NI - 1),
                    )

            def emit_finalize(j):
                js = slice(j * NT, (j + 1) * NT)
                # 1/S for the 8 heads (S at col 32h of the AV bank)
                half = 256 * (j % 2)
                acc = lpbig[:, 6, half : half + 256].rearrange(
                    "p (a b) -> p a b", b=32)
                rrec = finp.tile([128, 8, 1], F32, tag="rrec", name="rrec")
                nc.vector.reciprocal(out=rrec, in_=acc[:, :, 0:1])
                # attT[n, 17h+t] = relu(acc[n, 32h+t]) * rrec[n, h]
                attT = finp.tile([128, 8, 17], BF16, tag="attT", name="attT")
                nc.vector.scalar_tensor_tensor(
                    out=attT, in0=acc[:, :, 0:17], scalar=0.0,
                    in1=rrec.broadcast_to([128, 8, 17]),
                    op0=ALU.max, op1=ALU.mult,
                )
                # transpose to channel-major: two [128,68] -> [68,128] into bank7
                attF = attT.rearrange("p a b -> p (a b)")
                nc.tensor.transpose(
                    out=lpb16[0:68, 7, 0:128], in_=attF[:, 0:68], identity=idn)
                nc.tensor.transpose(
                    out=lpb16[0:68, 7, 128:256], in_=attF[:, 68:136], identity=idn)
                projin = finp.tile([68, 2, 128], BF16, tag="projin", name="projin")
                drain_copy(projin,
                           lpb16[0:68, 7, 0:256].rearrange("p (a b) -> p a b", a=2))
                # 1x1 proj (no bias): contraction over the 136 channels in 2 chunks
                for c in range(2):
                    nc.tensor.matmul(
                        out=lpbig[:, 7, 128:256],
                        lhsT=wpj[:, c, :], rhs=projin[:, c, :],
                        start=(c == 0), stop=(c == 1),
                    )
                drain_copy(o2[:, js], lpbig[:, 7, 128:256])
                nc.vector.tensor_reduce(
                    out=s1p[:, j : j + 1], in_=o2[:, js],
                    op=ALU.add, axis=mybir.AxisListType.X,
                )
                nc.vector.scalar_tensor_tensor(
                    out=sqd[:, js], in0=o2[:, js], scalar=1.0, in1=o2[:, js],
                    op0=ALU.mult, op1=ALU.mult, accum_out=s2p[:, j : j + 1],
                )

            NU = len(units)
            pt_hold = deque()
            for u in range(NU + 2):
                if u < NI:
                    emit_vt(u)
                if u < NU:
                    emit_logits(u)
                if 0 <= u - 1 < NU:
                    pt_hold.append(emit_exp(u - 1))
                if 0 <= u - 2 < NU:
                    emit_av(u - 2, pt_hold.popleft())
                    jj_, ii_ = units[u - 2]
                    if ii_ == NI - 1:
                        emit_finalize(jj_)

            # ---- GroupNorm ----
            s12 = pp.tile([DIM, 2], F32, tag="s12")
            nc.vector.tensor_reduce(
                out=s12[:, 0:1], in_=s1p, op=ALU.add, axis=mybir.AxisListType.X
            )
            nc.vector.tensor_reduce(
                out=s12[:, 1:2], in_=s2p, op=ALU.add, axis=mybir.AxisListType.X
            )
            s12r = pp.tile([DIM, 2], F32R, tag="s12r")
            nc.vector.tensor_copy(out=s12r, in_=s12)
            gselr = pp.tile([DIM, 8], F32R, tag="gselr")
            nc.vector.tensor_copy(out=gselr, in_=gsel)
            gp = psum_bank()
            nc.tensor.matmul(
                out=gp[0:8, 0, 0:2], lhsT=gselr, rhs=s12r, start=True, stop=True
            )
            gst = pp.tile([8, 2], F32, tag="gst")
            nc.vector.tensor_copy(out=gst, in_=gp[0:8, 0, 0:2])
            ccw = nc.gpsimd.dma_start(out=cc_in[:, :], in_=gst)
            if with_cc:
                cci = nc.gpsimd.collective_compute(
                    "AllReduce", ALU.add,
                    ins=[cc_in[:, :]], outs=[cc_out[:, :]],
                    replica_groups=[[0, 1], [2, 3], [4, 5], [6, 7]],
                )
            else:
                cci = nc.gpsimd.dma_start(out=cc_out[:, :], in_=cc_in[:, :])
            add_dep_helper(cci.ins, ccw.ins, reason="cc_in RAW")
            gch = pp.tile([DIM, 2], F32, tag="gch")
            ccr = nc.gpsimd.dma_start(
                out=gch,
                in_=bass.AP(
                    tensor=cc_out[:, :].tensor, offset=0,
                    ap=[[2, 8], [0, 16], [1, 2]],
                ),
            )
            add_dep_helper(ccr.ins, cci.ins, reason="cc_out RAW")
            # mu, var -> rstd = exp(-0.5*ln(var+eps)); A = rstd*gamma;
            # Bc = beta - mu*A; out = o2*A + Bc
            mu = pp.tile([DIM, 1], F32, tag="mu")
            nc.vector.tensor_scalar_mul(out=mu, in0=gch[:, 0:1], scalar1=GN_DIV)
            ex2 = pp.tile([DIM, 1], F32, tag="ex2")
            nc.vector.tensor_scalar_mul(out=ex2, in0=gch[:, 1:2], scalar1=GN_DIV)
            mu2 = pp.tile([DIM, 1], F32, tag="mu2")
            nc.vector.tensor_mul(out=mu2, in0=mu, in1=mu)
            var = pp.tile([DIM, 1], F32, tag="var")
            nc.vector.tensor_sub(out=var, in0=ex2, in1=mu2)
            epst = pp.tile([DIM, 1], F32, tag="epst")
            nc.vector.memset(epst, EPS)
            lnv = pp.tile([DIM, 1], F32, tag="lnv")
            nc.scalar.activation(out=lnv, in_=var, func=AF.Ln, bias=epst)
            rstd = pp.tile([DIM, 1], F32, tag="rstd")
            nc.scalar.activation(out=rstd, in_=lnv, func=AF.Exp, scale=-0.5)
            A = pp.tile([DIM, 1], F32, tag="A")
            nc.vector.tensor_mul(out=A, in0=rstd, in1=gab[:, 0:1])
            muA = pp.tile([DIM, 1], F32, tag="muA")
            nc.vector.tensor_mul(out=muA, in0=mu, in1=A)
            Bc = pp.tile([DIM, 1], F32, tag="Bc")
            nc.vector.tensor_sub(out=Bc, in0=gab[:, 1:2], in1=muA)
            of = pp.tile([DIM, NSL], F32, tag="of")
            nc.gpsimd.tensor_scalar(
                out=of, in0=o2, scalar1=A, scalar2=Bc,
                op0=ALU.mult, op1=ALU.add,
            )
            nc.sync.dma_start(out=out_d[:, :], in_=of)

    _split_multi_waits(nc)
    return nc


_CACHE = {}


def _prep(w_qkv, b_qkv, w_dw, b_dw, w_proj, gn_w, gn_b):
    """Host-side weight layout prep. head h = 4g+jj, channel ch = 16h+d."""
    ch = lambda h, d: 16 * h + d
    wk = np.zeros((2, DIM, 128), np.float32)
    w2 = np.zeros((2, 9, DIM, 128), np.float32)
    bq = np.zeros((2, 128), np.float32)
    wv = np.zeros((DIM, 136), np.float32)
    bvr = np.zeros((128, 136), np.float32)
    wpj = np.zeros((68, 2, 128), np.float32)
    dwsum = w_dw[:, 0].sum(axis=(1, 2))  # [128]
    for g in range(2):
        for jj in range(4):
            h = 4 * g + jj
            for d in range(16):
                c = ch(h, d)
                p = 32 * jj + d
                wk[g, :, p] = w_qkv[128 + c, :]
                bq[g, p] = b_qkv[c] * dwsum[c] + b_dw[c]
                for tap in range(9):
                    ty, tx = tap // 3, tap % 3
                    w2[g, tap, :, p] = w_dw[c, 0, ty, tx] * w_qkv[c, :]
    for h in range(HEADS):
        bvr[:, 17 * h] = 1.0
        for d in range(16):
            c = ch(h, d)
            wv[:, 17 * h + 1 + d] = w_qkv[256 + c, :]
            bvr[:, 17 * h + 1 + d] = b_qkv[256 + c]
            cp = 17 * h + 1 + d   # attT channel index
            wpj[cp % 68, cp // 68, :] = w_proj[:, c]
    gab = np.stack([gn_w, gn_b], axis=1).astype(np.float32)
    gsel = np.zeros((DIM, 8), np.float32)
    for c in range(DIM):
        gsel[c, c // 16] = 1.0
    idn = np.eye(DIM, 128, dtype=np.float32)
    # pad pixel x-vector: projects exactly to -b_q so biased q is 0 there
    vpad = -np.linalg.solve(w_qkv[0:128, :].astype(np.float64),
                            b_qkv[0:128].astype(np.float64)).astype(np.float32)
    weights = dict(
        wk=wk.transpose(1, 0, 2).astype(NPBF16),
        w2=w2.reshape(18, DIM, 128).transpose(1, 0, 2).astype(NPBF16),
        bq=bq.T.copy(),
        wv=wv.astype(NPBF16), bvr=bvr.astype(NPBF16),
        wpj=wpj.astype(NPBF16), idn=idn.astype(NPBF16),
        gab=gab, gsel=gsel,
    )
    return weights, vpad


def kernel(x, w_qkv, b_qkv, w_dw, b_dw, w_proj, gn_w, gn_b):
    x = np.asarray(x, np.float32)
    w_qkv = np.asarray(w_qkv, np.float32)
    b_qkv = np.asarray(b_qkv, np.float32)
    w_dw = np.asarray(w_dw, np.float32)
    b_dw = np.asarray(b_dw, np.float32)
    w_proj = np.asarray(w_proj, np.float32)
    gn_w = np.asarray(gn_w, np.float32)
    gn_b = np.asarray(gn_b, np.float32)

    weights, vpad = _prep(w_qkv, b_qkv, w_dw, b_dw, w_proj, gn_w, gn_b)

    if "nc" not in _CACHE:
        _CACHE["nc"] = _build()
    nc = _CACHE["nc"]

    in_maps = []
    for c in range(8):
        b, s = c // 2, c % 2
        xb = x[b].reshape(DIM, N)
        # q source: image rows 24s-1 .. 24s+24 with vpad padding (rows and
        # cols) so the post-projection-biased q is exactly 0 on the halo
        xq = np.empty((DIM, 26, 50), np.float32)
        xq[:, :, :] = vpad[:, None, None]
        xv = x[b]  # [DIM, H, W]
        if s == 0:
            xq[:, 1:26, 1:49] = xv[:, 0:25, :]
        else:
            xq[:, 0:25, 1:49] = xv[:, 23:48, :]
        m = {"xb": np.ascontiguousarray(xb).astype(NPBF16),
             "xq": xq.reshape(DIM, 26 * 50).astype(NPBF16)}
        m.update(weights)
        in_maps.append(m)

    res = run_bass_kernel_spmd(nc, in_maps, core_ids=list(range(8)))

    out = np.empty((B, DIM, H, W), np.float32)
    for c in range(8):
        b, s = c // 2, c % 2
        out[b, :, 24 * s : 24 * s + 24, :] = res.results[c]["out_half"].reshape(
            DIM, ROWS_HALF, W
        )
    return out


# revision 9
# speedup vs baseline: 1.0133x; 1.0133x over previous
"""Trainium2 Bass kernel for nn_Attention_44830868635854.

Fused: 1x1-conv QKV -> depthwise 3x3 on q -> 8-head attention (softmax) ->
ReLU -> 1x1 proj -> GroupNorm(8).

Sharding: 8 cores = (batch b in 0..3) x (spatial half s in 0..1). Each core
computes output rows [24s, 24s+24) of the 48x48 image for its batch (1152
query pixels) across all 8 heads, using the full image for k/v. GroupNorm
statistics are combined across the core pair with a tiny AllReduce.

Main-loop structure (per core), NT=128 query tile, MT=128 key tile:
  unit u=(j,i): 8 logit matmuls (bf16 q stream vs f32r k weights) into a
  rotating 2-bank PSUM slot (3 slots); exp of the [128,1024] logit block on
  either the ACT engine (native Exp -> bf16) or the DVE engine (custom
  single-instruction quartic-polynomial exp, ~1e-3 rel err) -- the exp
  elementwise work is the kernel's roofline so it is split across both
  engines; AV uses P as the PE *weights* (lhsT) with a tiny [128,17] v/ones
  rhs, accumulating O^T and the softmax denominator S in PSUM across all 18
  key tiles. Finalize per j: reciprocal of S, fused relu+normalize
  (scalar_tensor_tensor), PE transpose back to channel-major, 1x1 proj, and
  GroupNorm partial sums.
"""

from collections import deque

import numpy as np
import ml_dtypes

import concourse.bass as bass
import concourse.mybir as mybir
import concourse.tile as tile
from concourse.tile import add_dep_helper
from concourse.bass_utils import run_bass_kernel_spmd
from concourse import dve_ops as _DO
from concourse.dve_spec import Spec, Src0, Src1, C0, C1, C2, One, lower as _dve_lower
from concourse.dve_uop import DveOpSpec as _DveOpSpec

F32 = mybir.dt.float32
F32R = mybir.dt.float32r
BF16 = mybir.dt.bfloat16
AF = mybir.ActivationFunctionType
ALU = mybir.AluOpType
NPBF16 = ml_dtypes.bfloat16

B, DIM, H, W = 4, 128, 48, 48
HEADS, HEAD_DIM = 8, 16
N = H * W            # 2304
ROWS_HALF = 24
NSL = ROWS_HALF * W  # 1152 per core
NT = 128             # query tile (9 per core)
MT = 128             # key tile (18 per core)
NJ = NSL // NT       # 9
NI = N // MT         # 18
EPS = 1e-5
GN_DIV = 1.0 / (16.0 * N)

# quartic exp(L/4) fit over L in [-4.4, 4.3]: max rel err 9.5e-4
# P(L) = (((L*K4 + K3)*L + K2)*L + K1)*L + 1
K4 = 0.00015327319036728373
K3 = 0.002763773359872127
K2 = 0.03147120315761681
K1 = 0.24957119869968478

# exp engine split: DVE for these u%13 slots (4/13 ~ 50 of 162 units)
EXP_DVE_MOD = 13
EXP_DVE_PAT = (3, 7, 11, 12)


def _register_exp4():
    """Register a custom single-instruction DVE op computing the quartic
    exp approximation (Horner, P(0)=1 exact). Data-driven: opcode row +
    per-NEFF uop table, same mechanism as the stock custom DVE ops."""
    name = "EXP_POLY4_ANT"
    if name in _DO._SUB_OPCODE_FOR_NAME:
        for op in _DO.OPS:
            if op.name == name:
                return op
    body = (((Src0 * C0 + C1) * Src0 + C2) * Src0 + Src1) * Src0 + One

    def ref(in0, in1, c0, c1, c2):
        x = in0.astype(np.float32)
        return ((((x * c0 + c1) * x + c2) * x + in1) * x + 1.0).astype(np.float32)

    spec = Spec(body=body, reference=ref)
    row = _DO._CUSTOM_DVE_ROW_BASE + len(_DO.OPS)
    assert row < 0x20
    shas = {}
    for ver in ("v3", "v4"):
        s = _DveOpSpec(name=name, opcode=row, uops=_dve_lower(spec, ver=ver),
                       rd1_en=True)
        shas[ver] = s.sha(ver)
    op = _DO.DveOp(name, spec, subdim=False, uops_sha=shas)
    _DO.OPS.append(op)
    _DO.CUSTOM_DVE_SPECS[name] = spec
    _DO._SUB_OPCODE_FOR_NAME[name] = row
    return op


EXP4 = _register_exp4()


def _split_multi_waits(nc):
    """walrus here allows one sync-wait slot per lowered instruction; move
    extra waits onto standalone EventSemaphore instructions."""
    for func in nc.m.functions:
        for block in func.blocks:
            new_insts = []
            for inst in block.instructions:
                si = inst.sync_info
                waits = list(si.on_wait) if si is not None and si.on_wait else []
                if len(waits) > 1 and not isinstance(inst, mybir.InstEventSemaphore):
                    for k, w in enumerate(waits[:-1]):
                        new_insts.append(
                            mybir.InstEventSemaphore(
                                name=f"{inst.name}_wsplit{k}",
                                engine=inst.engine,
                                ins=[],
                                outs=[],
                                sync_info=mybir.SyncInfo(on_wait=[w], on_update=[]),
                            )
                        )
                    si.on_wait = waits[-1:]
                new_insts.append(inst)
            block.instructions[:] = new_insts


def _build(with_cc=True):
    nc = bass.Bass()
    dt = nc.dram_tensor

    xb_d = dt("xb", [DIM, N], BF16, kind="ExternalInput")
    xq_d = dt("xq", [DIM, 26 * 50], BF16, kind="ExternalInput")
    wk_d = dt("wk", [DIM, 2, 128], BF16, kind="ExternalInput")
    w2_d = dt("w2", [DIM, 18, 128], BF16, kind="ExternalInput")
    bq_d = dt("bq", [128, 2], F32, kind="ExternalInput")
    wv_d = dt("wv", [DIM, 136], BF16, kind="ExternalInput")
    bvr_d = dt("bvr", [128, 136], BF16, kind="ExternalInput")
    wpj_d = dt("wpj", [68, 2, 128], BF16, kind="ExternalInput")
    idn_d = dt("idn", [DIM, 128], BF16, kind="ExternalInput")
    gab_d = dt("gab", [DIM, 2], F32, kind="ExternalInput")  # gn gamma | beta
    gsel_d = dt("gsel", [DIM, 8], F32, kind="ExternalInput")

    out_d = dt("out_half", [DIM, NSL], F32, kind="ExternalOutput")

    cc_in = dt("cc_in", [8, 2], F32)
    cc_out = dt("cc_out", [8, 2], F32)
    scratch_d = dt("scratch", [128, 1], F32)

    with tile.TileContext(nc) as tc:
        with (
            tc.tile_pool(name="persist", bufs=1) as pp,
            tc.tile_pool(name="ptp", bufs=3) as ptp,
            tc.tile_pool(name="fin", bufs=2) as finp,
            tc.tile_pool(name="lp", bufs=1, space="PSUM") as lpp,
        ):
            lpbig = lpp.tile([128, 8, 512], F32, tag="lpbig")
            lpb16 = lpbig.bitcast(BF16)  # [128, 8, 1024]
            psum_rr = [0]

            def psum_bank():
                b = psum_rr[0] % 8
                psum_rr[0] += 1
                return lpbig[:, b : b + 1, :]

            # drain engine alternation: weighted ACT/DVE copy
            drain_rr = [0]

            def drain_copy(out, in_):
                drain_rr[0] += 1
                if drain_rr[0] % 2 == 0:
                    nc.scalar.copy(out=out, in_=in_)
                else:
                    nc.vector.tensor_copy(out=out, in_=in_)

            # ---- ACT exp table preload (single-wait discipline for hot loop)
            dummy = pp.tile([128, 1], F32, tag="dummy")
            nc.vector.memset(dummy, 0.0)
            nc.scalar.activation(out=dummy, in_=dummy, func=AF.Exp)
            nc.gpsimd.dma_start(out=scratch_d[:, :], in_=dummy)

            # exp4 k1 constant (per-partition broadcast source)
            k1t = pp.tile([128, 1], F32, tag="k1t")
            nc.gpsimd.memset(k1t, K1)

            # ---- load inputs (pre-formatted on host; no on-device conversion)
            xb = pp.tile([DIM, N], BF16, tag="xb")
            nc.sync.dma_start(out=xb, in_=xb_d[:, :])
            xq = pp.tile([DIM, 26 * 50], BF16, tag="xq")
            nc.sync.dma_start(out=xq, in_=xq_d[:, :])
            wkt = pp.tile([DIM, 2, 128], BF16, tag="wkt")
            nc.sync.dma_start(out=wkt, in_=wk_d[:, :, :])
            wk = [wkt[:, g, :] for g in range(2)]
            w2t = pp.tile([DIM, 18, 128], BF16, tag="w2t")
            nc.sync.dma_start(out=w2t, in_=w2_d[:, :, :])
            w2 = [w2t[:, i, :] for i in range(18)]
            bqt = pp.tile([128, 2], F32, tag="bqt")
            nc.sync.dma_start(out=bqt, in_=bq_d[:, :])
            bqv = [bqt[:, g : g + 1] for g in range(2)]
            wv = pp.tile([DIM, 136], BF16, tag="wv")
            nc.sync.dma_start(out=wv, in_=wv_d[:, :])
            bvr = pp.tile([128, 136], BF16, tag="bvr")
            nc.sync.dma_start(out=bvr, in_=bvr_d[:, :])
            wpj = pp.tile([68, 2, 128], BF16, tag="wpj")
            nc.sync.dma_start(out=wpj, in_=wpj_d[:, :, :])
            idn = pp.tile([DIM, 128], BF16, tag="idn")
            nc.sync.dma_start(out=idn, in_=idn_d[:, :])
            gab = pp.tile([DIM, 2], F32, tag="gab")
            nc.sync.dma_start(out=gab, in_=gab_d[:, :])
            gsel = pp.tile([DIM, 8], F32, tag="gsel")
            nc.sync.dma_start(out=gsel, in_=gsel_d[:, :])

            # ---- QKV projections ----
            kg = [None, None]
            qg = [None, None]

            def emit_k(g):
                kt = pp.tile([DIM, N], F32R, tag=f"kg{g}", name=f"kg{g}")
                for j0 in range(0, N, 512):
                    n = min(512, N - j0)
                    ps = psum_bank()
                    nc.tensor.matmul(
                        out=ps[:, 0, 0:n], lhsT=wk[g], rhs=xb[:, j0 : j0 + n],
                        start=True, stop=True,
                    )
                    drain_copy(kt[:, j0 : j0 + n], ps[:, 0, 0:n])
                kg[g] = kt

            qg[0] = pp.tile([128, NSL], BF16, tag="qg0", name="qg0")
            qg[1] = pp.tile([128, NSL], BF16, tag="qg1", name="qg1")

            def emit_q_block(g, blk, ps):
                xqv = xq.rearrange("p (r c) -> p r c", c=50)
                for ty in range(3):
                    for tx in range(3):
                        tap = 3 * ty + tx
                        nc.tensor.matmul(
                            out=ps[:, 0, 0:384],
                            lhsT=w2[9 * g + tap],
                            rhs=xqv[:, 8 * blk + ty : 8 * blk + ty + 8,
                                    tx : tx + W],
                            start=(tap == 0), stop=(tap == 8),
                        )
                nc.vector.tensor_scalar_add(
                    out=qg[g][:, blk * 384 : (blk + 1) * 384],
                    in0=ps[:, 0, 0:384],
                    scalar1=bqv[g],
                )

            vt = pp.tile([128, NI, 136], BF16, tag="vt")

            def emit_vt(i):
                # bank 7 halves; free during the first j-tile's units
                col = 256 * (i % 2)
                nc.tensor.matmul(
                    out=lpbig[:, 7, col : col + 136],
                    lhsT=xb[:, i * MT : (i + 1) * MT],
                    rhs=wv, start=True, stop=True,
                )
                if i % 2 == 1:
                    pair = lpbig[:, 7, :].rearrange(
                        "p (a b) -> p a b", b=256)[:, :, 0:136]
                    nc.vector.tensor_add(
                        out=vt[:, i - 1 : i + 1, :], in0=pair,
                        in1=bvr.rearrange("p (a b) -> p a b", a=1
                                          ).broadcast_to([128, 2, 136]),
                    )

            emit_k(0)
            emit_k(1)
            emit_q_block(0, 0, psum_bank())
            emit_q_block(1, 0, psum_bank())

            # ---- attention main loop ----
            o2 = pp.tile([DIM, NSL], BF16, tag="o2")
            sqd = pp.tile([DIM, NSL], BF16, tag="sqd")
            s1p = pp.tile([DIM, NJ], F32, tag="s1p")
            s2p = pp.tile([DIM, NJ], F32, tag="s2p")
            units = [(j, i) for j in range(NJ) for i in range(NI)]

            def emit_logits(u):
                j, i = units[u]
                z = u % 3
                js = slice(j * NT, (j + 1) * NT)
                for g in range(2):
                    for jj in range(4):
                        h = 4 * g + jj
                        nc.tensor.matmul(
                            out=lpbig[:, 2 * z + h // 4,
                                      128 * (h % 4) : 128 * (h % 4) + 128],
                            lhsT=kg[g][32 * jj : 32 * jj + 16,
                                       i * MT : (i + 1) * MT],
                            rhs=qg[g][32 * jj : 32 * jj + 16, js],
                            start=True, stop=True,
                            tile_position=(32 * jj, 0),
                        )

            def emit_exp(u):
                z = u % 3
                src = lpbig[:, 2 * z : 2 * z + 2, :].rearrange("p a b -> p (a b)")
                pt = ptp.tile([128, 8, 128], BF16, tag="pt", name="pt")
                flat = pt.rearrange("p a b -> p (a b)")
                if (u % EXP_DVE_MOD) in EXP_DVE_PAT:
                    nc.vector._custom_dve(
                        EXP4, out=flat, in0=src,
                        in1=k1t[:, 0:1].broadcast_to([128, 1024]),
                        s0=K4, s1=K3, imm2=K2,
                    )
                else:
                    nc.scalar.activation(out=flat, in_=src, func=AF.Exp, scale=0.25)
                return pt

            def emit_av(u, pt):
                j, i = units[u]
                half = 256 * (j % 2)  # double-buffered AV accumulator
                for h in range(HEADS):
                    nc.tensor.matmul(
                        out=lpbig[:, 6, half + 32 * h : half + 32 * h + 17],
                        lhsT=pt[:, h, :],
                        rhs=vt[:, i, 17 * h : 17 * h + 17],
                        start=(i == 0), stop=(i == NI - 1),
                    )

            def emit_finalize(j):
                js = slice(j * NT, (j + 1) * NT)
                # 1/S for the 8 heads (S at col 32h of the AV bank)
                half = 256 * (j % 2)
                acc = lpbig[:, 6, half : half + 256].rearrange(
                    "p (a b) -> p a b", b=32)
                rrec = finp.tile([128, 8, 1], F32, tag="rrec", name="rrec")
                nc.vector.reciprocal(out=rrec, in_=acc[:, :, 0:1])
                # attT[n, 17h+t] = relu(acc[n, 32h+t]) * rrec[n, h]
                attT = finp.tile([128, 8, 17], BF16, tag="attT", name="attT")
                nc.vector.scalar_tensor_tensor(
                    out=attT, in0=acc[:, :, 0:17], scalar=0.0,
                    in1=rrec.broadcast_to([128, 8, 17]),
                    op0=ALU.max, op1=ALU.mult,
                )
                # transpose to channel-major: two [128,68] -> [68,128] into bank7
                attF = attT.rearrange("p a b -> p (a b)")
                nc.tensor.transpose(
                    out=lpb16[0:68, 7, 0:128], in_=attF[:, 0:68], identity=idn)
                nc.tensor.transpose(
                    out=lpb16[0:68, 7, 128:256], in_=attF[:, 68:136], identity=idn)
                projin = finp.tile([68, 2, 128], BF16, tag="projin", name="projin")
                drain_copy(projin,
                           lpb16[0:68, 7, 0:256].rearrange("p (a b) -> p a b", a=2))
                # 1x1 proj (no bias): contraction over the 136 channels in 2 chunks
                for c in range(2):
                    nc.tensor.matmul(
                        out=lpbig[:, 7, 128:256],
                        lhsT=wpj[:, c, :], rhs=projin[:, c, :],
                        start=(c == 0), stop=(c == 1),
                    )
                drain_copy(o2[:, js], lpbig[:, 7, 128:256])
                nc.vector.tensor_reduce(
                    out=s1p[:, j : j + 1], in_=o2[:, js],
                    op=ALU.add, axis=mybir.AxisListType.X,
                )
                nc.vector.scalar_tensor_tensor(
                    out=sqd[:, js], in0=o2[:, js], scalar=1.0, in1=o2[:, js],
                    op0=ALU.mult, op1=ALU.mult, accum_out=s2p[:, j : j + 1],
                )

            NU = len(units)
            pt_hold = deque()
            qlate = [(0, 1), (1, 1), (0, 2), (1, 2)]
            for u in range(NU + 2):
                if u < NI:
                    emit_vt(u)
                elif NI <= u < NI + 4:
                    g_, b_ = qlate[u - NI]
                    emit_q_block(g_, b_, lpbig[:, 7:8, :])
                if u < NU:
                    emit_logits(u)
                if 0 <= u - 1 < NU:
                    pt_hold.append(emit_exp(u - 1))
                if 0 <= u - 2 < NU:
                    emit_av(u - 2, pt_hold.popleft())
                    jj_, ii_ = units[u - 2]
                    if ii_ == NI - 1:
                        emit_finalize(jj_)

            # ---- GroupNorm ----
            s12 = pp.tile([DIM, 2], F32, tag="s12")
            nc.vector.tensor_reduce(
                out=s12[:, 0:1], in_=s1p, op=ALU.add, axis=mybir.AxisListType.X
            )
            nc.vector.tensor_reduce(
                out=s12[:, 1:2], in_=s2p, op=ALU.add, axis=mybir.AxisListType.X
            )
            s12r = pp.tile([DIM, 2], F32R, tag="s12r")
            nc.vector.tensor_copy(out=s12r, in_=s12)
            gselr = pp.tile([DIM, 8], F32R, tag="gselr")
            nc.vector.tensor_copy(out=gselr, in_=gsel)
            gp = psum_bank()
            nc.tensor.matmul(
                out=gp[0:8, 0, 0:2], lhsT=gselr, rhs=s12r, start=True, stop=True
            )
            gst = pp.tile([8, 2], F32, tag="gst")
            nc.vector.tensor_copy(out=gst, in_=gp[0:8, 0, 0:2])
            ccw = nc.gpsimd.dma_start(out=cc_in[:, :], in_=gst)
            if with_cc:
                cci = nc.gpsimd.collective_compute(
                    "AllReduce", ALU.add,
                    ins=[cc_in[:, :]], outs=[cc_out[:, :]],
                    replica_groups=[[0, 1], [2, 3], [4, 5], [6, 7]],
                )
            else:
                cci = nc.gpsimd.dma_start(out=cc_out[:, :], in_=cc_in[:, :])
            add_dep_helper(cci.ins, ccw.ins, reason="cc_in RAW")
            gch = pp.tile([DIM, 2], F32, tag="gch")
            ccr = nc.gpsimd.dma_start(
                out=gch,
                in_=bass.AP(
                    tensor=cc_out[:, :].tensor, offset=0,
                    ap=[[2, 8], [0, 16], [1, 2]],
                ),
            )
            add_dep_helper(ccr.ins, cci.ins, reason="cc_out RAW")
            # mu, var -> rstd = exp(-0.5*ln(var+eps)); A = rstd*gamma;
            # Bc = beta - mu*A; out = o2*A + Bc
            mu = pp.tile([DIM, 1], F32, tag="mu")
            nc.vector.tensor_scalar_mul(out=mu, in0=gch[:, 0:1], scalar1=GN_DIV)
            ex2 = pp.tile([DIM, 1], F32, tag="ex2")
            nc.vector.tensor_scalar_mul(out=ex2, in0=gch[:, 1:2], scalar1=GN_DIV)
            mu2 = pp.tile([DIM, 1], F32, tag="mu2")
            nc.vector.tensor_mul(out=mu2, in0=mu, in1=mu)
            var = pp.tile([DIM, 1], F32, tag="var")
            nc.vector.tensor_sub(out=var, in0=ex2, in1=mu2)
            epst = pp.tile([DIM, 1], F32, tag="epst")
            nc.vector.memset(epst, EPS)
            lnv = pp.tile([DIM, 1], F32, tag="lnv")
            nc.scalar.activation(out=lnv, in_=var, func=AF.Ln, bias=epst)
            rstd = pp.tile([DIM, 1], F32, tag="rstd")
            nc.scalar.activation(out=rstd, in_=lnv, func=AF.Exp, scale=-0.5)
            A = pp.tile([DIM, 1], F32, tag="A")
            nc.vector.tensor_mul(out=A, in0=rstd, in1=gab[:, 0:1])
            muA = pp.tile([DIM, 1], F32, tag="muA")
            nc.vector.tensor_mul(out=muA, in0=mu, in1=A)
            Bc = pp.tile([DIM, 1], F32, tag="Bc")
            nc.vector.tensor_sub(out=Bc, in0=gab[:, 1:2], in1=muA)
            of = pp.tile([DIM, NSL], F32, tag="of")
            nc.gpsimd.tensor_scalar(
                out=of, in0=o2, scalar1=A, scalar2=Bc,
                op0=ALU.mult, op1=ALU.add,
            )
            nc.sync.dma_start(out=out_d[:, :], in_=of)

    _split_multi_waits(nc)
    return nc


_CACHE = {}


def _prep(w_qkv, b_qkv, w_dw, b_dw, w_proj, gn_w, gn_b):
    """Host-side weight layout prep. head h = 4g+jj, channel ch = 16h+d."""
    ch = lambda h, d: 16 * h + d
    wk = np.zeros((2, DIM, 128), np.float32)
    w2 = np.zeros((2, 9, DIM, 128), np.float32)
    bq = np.zeros((2, 128), np.float32)
    wv = np.zeros((DIM, 136), np.float32)
    bvr = np.zeros((128, 136), np.float32)
    wpj = np.zeros((68, 2, 128), np.float32)
    dwsum = w_dw[:, 0].sum(axis=(1, 2))  # [128]
    for g in range(2):
        for jj in range(4):
            h = 4 * g + jj
            for d in range(16):
                c = ch(h, d)
                p = 32 * jj + d
                wk[g, :, p] = w_qkv[128 + c, :]
                bq[g, p] = b_qkv[c] * dwsum[c] + b_dw[c]
                for tap in range(9):
                    ty, tx = tap // 3, tap % 3
                    w2[g, tap, :, p] = w_dw[c, 0, ty, tx] * w_qkv[c, :]
    for h in range(HEADS):
        bvr[:, 17 * h] = 1.0
        for d in range(16):
            c = ch(h, d)
            wv[:, 17 * h + 1 + d] = w_qkv[256 + c, :]
            bvr[:, 17 * h + 1 + d] = b_qkv[256 + c]
            cp = 17 * h + 1 + d   # attT channel index
            wpj[cp % 68, cp // 68, :] = w_proj[:, c]
    gab = np.stack([gn_w, gn_b], axis=1).astype(np.float32)
    gsel = np.zeros((DIM, 8), np.float32)
    for c in range(DIM):
        gsel[c, c // 16] = 1.0
    idn = np.eye(DIM, 128, dtype=np.float32)
    # pad pixel x-vector: projects exactly to -b_q so biased q is 0 there
    vpad = -np.linalg.solve(w_qkv[0:128, :].astype(np.float64),
                            b_qkv[0:128].astype(np.float64)).astype(np.float32)
    weights = dict(
        wk=wk.transpose(1, 0, 2).astype(NPBF16),
        w2=w2.reshape(18, DIM, 128).transpose(1, 0, 2).astype(NPBF16),
        bq=bq.T.copy(),
        wv=wv.astype(NPBF16), bvr=bvr.astype(NPBF16),
        wpj=wpj.astype(NPBF16), idn=idn.astype(NPBF16),
        gab=gab, gsel=gsel,
    )
    return weights, vpad


def kernel(x, w_qkv, b_qkv, w_dw, b_dw, w_proj, gn_w, gn_b):
    x = np.asarray(x, np.float32)
    w_qkv = np.asarray(w_qkv, np.float32)
    b_qkv = np.asarray(b_qkv, np.float32)
    w_dw = np.asarray(w_dw, np.float32)
    b_dw = np.asarray(b_dw, np.float32)
    w_proj = np.asarray(w_proj, np.float32)
    gn_w = np.asarray(gn_w, np.float32)
    gn_b = np.asarray(gn_b, np.float32)

    weights, vpad = _prep(w_qkv, b_qkv, w_dw, b_dw, w_proj, gn_w, gn_b)

    if "nc" not in _CACHE:
        _CACHE["nc"] = _build()
    nc = _CACHE["nc"]

    in_maps = []
    for c in range(8):
        b, s = c // 2, c % 2
        xb = x[b].reshape(DIM, N)
        # q source: image rows 24s-1 .. 24s+24 with vpad padding (rows and
        # cols) so the post-projection-biased q is exactly 0 on the halo
        xq = np.empty((DIM, 26, 50), np.float32)
        xq[:, :, :] = vpad[:, None, None]
        xv = x[b]  # [DIM, H, W]
        if s == 0:
            xq[:, 1:26, 1:49] = xv[:, 0:25, :]
        else:
            xq[:, 0:25, 1:49] = xv[:, 23:48, :]
        m = {"xb": np.ascontiguousarray(xb).astype(NPBF16),
             "xq": xq.reshape(DIM, 26 * 50).astype(NPBF16)}
        m.update(weights)
        in_maps.append(m)

    res = run_bass_kernel_spmd(nc, in_maps, core_ids=list(range(8)))

    out = np.empty((B, DIM, H, W), np.float32)
    for c in range(8):
        b, s = c // 2, c % 2
        out[b, :, 24 * s : 24 * s + 24, :] = res.results[c]["out_half"].reshape(
            DIM, ROWS_HALF, W
        )
    return out


# revision 10
# speedup vs baseline: 1.0216x; 1.0082x over previous
"""Trainium2 Bass kernel for nn_Attention_44830868635854.

Fused: 1x1-conv QKV -> depthwise 3x3 on q -> 8-head attention (softmax) ->
ReLU -> 1x1 proj -> GroupNorm(8).

Sharding: 8 cores = (batch b in 0..3) x (spatial half s in 0..1). Each core
computes output rows [24s, 24s+24) of the 48x48 image for its batch (1152
query pixels) across all 8 heads, using the full image for k/v. GroupNorm
statistics are combined across the core pair with a tiny AllReduce.

Main-loop structure (per core), NT=128 query tile, MT=128 key tile:
  unit u=(j,i): 8 logit matmuls (bf16 q stream vs f32r k weights) into a
  rotating 2-bank PSUM slot (3 slots); exp of the [128,1024] logit block on
  either the ACT engine (native Exp -> bf16) or the DVE engine (custom
  single-instruction quartic-polynomial exp, ~1e-3 rel err) -- the exp
  elementwise work is the kernel's roofline so it is split across both
  engines; AV uses P as the PE *weights* (lhsT) with a tiny [128,17] v/ones
  rhs, accumulating O^T and the softmax denominator S in PSUM across all 18
  key tiles. Finalize per j: reciprocal of S, fused relu+normalize
  (scalar_tensor_tensor), PE transpose back to channel-major, 1x1 proj, and
  GroupNorm partial sums.
"""

from collections import deque

import numpy as np
import ml_dtypes

import concourse.bass as bass
import concourse.mybir as mybir
import concourse.tile as tile
from concourse.tile import add_dep_helper
from concourse.bass_utils import run_bass_kernel_spmd
from concourse import dve_ops as _DO
from concourse.dve_spec import Spec, Src0, Src1, C0, C1, C2, One, lower as _dve_lower
from concourse.dve_uop import DveOpSpec as _DveOpSpec

F32 = mybir.dt.float32
F32R = mybir.dt.float32r
BF16 = mybir.dt.bfloat16
AF = mybir.ActivationFunctionType
ALU = mybir.AluOpType
NPBF16 = ml_dtypes.bfloat16

B, DIM, H, W = 4, 128, 48, 48
HEADS, HEAD_DIM = 8, 16
N = H * W            # 2304
ROWS_HALF = 24
NSL = ROWS_HALF * W  # 1152 per core
NT = 128             # query tile (9 per core)
MT = 128             # key tile (18 per core)
NJ = NSL // NT       # 9
NI = N // MT         # 18
EPS = 1e-5
GN_DIV = 1.0 / (16.0 * N)

# quartic exp(L/4) fit over L in [-4.4, 4.3]: max rel err 9.5e-4
# P(L) = (((L*K4 + K3)*L + K2)*L + K1)*L + 1
K4 = 0.00015327319036728373
K3 = 0.002763773359872127
K2 = 0.03147120315761681
K1 = 0.24957119869968478

# exp engine split: DVE for these u%13 slots (4/13 ~ 50 of 162 units)
EXP_DVE_MOD = 13
EXP_DVE_PAT = (3, 7, 11, 12)


def _register_exp4():
    """Register a custom single-instruction DVE op computing the quartic
    exp approximation (Horner, P(0)=1 exact). Data-driven: opcode row +
    per-NEFF uop table, same mechanism as the stock custom DVE ops."""
    name = "EXP_POLY4_ANT"
    if name in _DO._SUB_OPCODE_FOR_NAME:
        for op in _DO.OPS:
            if op.name == name:
                return op
    body = (((Src0 * C0 + C1) * Src0 + C2) * Src0 + Src1) * Src0 + One

    def ref(in0, in1, c0, c1, c2):
        x = in0.astype(np.float32)
        return ((((x * c0 + c1) * x + c2) * x + in1) * x + 1.0).astype(np.float32)

    spec = Spec(body=body, reference=ref)
    row = _DO._CUSTOM_DVE_ROW_BASE + len(_DO.OPS)
    assert row < 0x20
    shas = {}
    for ver in ("v3", "v4"):
        s = _DveOpSpec(name=name, opcode=row, uops=_dve_lower(spec, ver=ver),
                       rd1_en=True)
        shas[ver] = s.sha(ver)
    op = _DO.DveOp(name, spec, subdim=False, uops_sha=shas)
    _DO.OPS.append(op)
    _DO.CUSTOM_DVE_SPECS[name] = spec
    _DO._SUB_OPCODE_FOR_NAME[name] = row
    return op


EXP4 = _register_exp4()


def _split_multi_waits(nc):
    """walrus here allows one sync-wait slot per lowered instruction; move
    extra waits onto standalone EventSemaphore instructions."""
    for func in nc.m.functions:
        for block in func.blocks:
            new_insts = []
            for inst in block.instructions:
                si = inst.sync_info
                waits = list(si.on_wait) if si is not None and si.on_wait else []
                if len(waits) > 1 and not isinstance(inst, mybir.InstEventSemaphore):
                    for k, w in enumerate(waits[:-1]):
                        new_insts.append(
                            mybir.InstEventSemaphore(
                                name=f"{inst.name}_wsplit{k}",
                                engine=inst.engine,
                                ins=[],
                                outs=[],
                                sync_info=mybir.SyncInfo(on_wait=[w], on_update=[]),
                            )
                        )
                    si.on_wait = waits[-1:]
                new_insts.append(inst)
            block.instructions[:] = new_insts


def _build(with_cc=True):
    nc = bass.Bass()
    dt = nc.dram_tensor

    xb_d = dt("xb", [DIM, N], BF16, kind="ExternalInput")
    xq_d = dt("xq", [DIM, 26 * 50], BF16, kind="ExternalInput")
    wk_d = dt("wk", [DIM, 2, 128], BF16, kind="ExternalInput")
    w2_d = dt("w2", [DIM, 18, 128], BF16, kind="ExternalInput")
    bq_d = dt("bq", [128, 2], F32, kind="ExternalInput")
    wv_d = dt("wv", [DIM, 136], BF16, kind="ExternalInput")
    bvr_d = dt("bvr", [128, 136], BF16, kind="ExternalInput")
    wpj_d = dt("wpj", [68, 2, 128], BF16, kind="ExternalInput")
    idn_d = dt("idn", [DIM, 128], BF16, kind="ExternalInput")
    gab_d = dt("gab", [DIM, 2], F32, kind="ExternalInput")  # gn gamma | beta
    gsel_d = dt("gsel", [DIM, 8], F32, kind="ExternalInput")

    out_d = dt("out_half", [DIM, NSL], F32, kind="ExternalOutput")

    cc_in = dt("cc_in", [8, 2], F32)
    cc_out = dt("cc_out", [8, 2], F32)
    scratch_d = dt("scratch", [128, 1], F32)

    with tile.TileContext(nc) as tc:
        with (
            tc.tile_pool(name="persist", bufs=1) as pp,
            tc.tile_pool(name="ptp", bufs=6) as ptp,
            tc.tile_pool(name="fin", bufs=2) as finp,
            tc.tile_pool(name="lp", bufs=1, space="PSUM") as lpp,
        ):
            lpbig = lpp.tile([128, 8, 512], F32, tag="lpbig")
            lpb16 = lpbig.bitcast(BF16)  # [128, 8, 1024]
            psum_rr = [0]

            def psum_bank():
                b = psum_rr[0] % 8
                psum_rr[0] += 1
                return lpbig[:, b : b + 1, :]

            # drain engine alternation: weighted ACT/DVE copy
            drain_rr = [0]

            def drain_copy(out, in_):
                drain_rr[0] += 1
                if drain_rr[0] % 2 == 0:
                    nc.scalar.copy(out=out, in_=in_)
                else:
                    nc.vector.tensor_copy(out=out, in_=in_)

            # ---- ACT exp table preload (single-wait discipline for hot loop)
            dummy = pp.tile([128, 1], F32, tag="dummy")
            nc.vector.memset(dummy, 0.0)
            nc.scalar.activation(out=dummy, in_=dummy, func=AF.Exp)
            nc.gpsimd.dma_start(out=scratch_d[:, :], in_=dummy)

            # exp4 k1 constant (per-partition broadcast source)
            k1t = pp.tile([128, 1], F32, tag="k1t")
            nc.gpsimd.memset(k1t, K1)

            # ---- load inputs (pre-formatted on host; no on-device conversion)
            xb = pp.tile([DIM, N], BF16, tag="xb")
            nc.sync.dma_start(out=xb, in_=xb_d[:, :])
            xq = pp.tile([DIM, 26 * 50], BF16, tag="xq")
            nc.sync.dma_start(out=xq, in_=xq_d[:, :])
            wkt = pp.tile([DIM, 2, 128], BF16, tag="wkt")
            nc.sync.dma_start(out=wkt, in_=wk_d[:, :, :])
            wk = [wkt[:, g, :] for g in range(2)]
            w2t = pp.tile([DIM, 18, 128], BF16, tag="w2t")
            nc.sync.dma_start(out=w2t, in_=w2_d[:, :, :])
            w2 = [w2t[:, i, :] for i in range(18)]
            bqt = pp.tile([128, 2], F32, tag="bqt")
            nc.sync.dma_start(out=bqt, in_=bq_d[:, :])
            bqv = [bqt[:, g : g + 1] for g in range(2)]
            wv = pp.tile([DIM, 136], BF16, tag="wv")
            nc.sync.dma_start(out=wv, in_=wv_d[:, :])
            bvr = pp.tile([128, 136], BF16, tag="bvr")
            nc.sync.dma_start(out=bvr, in_=bvr_d[:, :])
            wpj = pp.tile([68, 2, 128], BF16, tag="wpj")
            nc.sync.dma_start(out=wpj, in_=wpj_d[:, :, :])
            idn = pp.tile([DIM, 128], BF16, tag="idn")
            nc.sync.dma_start(out=idn, in_=idn_d[:, :])
            gab = pp.tile([DIM, 2], F32, tag="gab")
            nc.sync.dma_start(out=gab, in_=gab_d[:, :])
            gsel = pp.tile([DIM, 8], F32, tag="gsel")
            nc.sync.dma_start(out=gsel, in_=gsel_d[:, :])

            # ---- QKV projections ----
            kg = [None, None]
            qg = [None, None]

            def emit_k(g):
                kt = pp.tile([DIM, N], F32R, tag=f"kg{g}", name=f"kg{g}")
                for j0 in range(0, N, 512):
                    n = min(512, N - j0)
                    ps = psum_bank()
                    nc.tensor.matmul(
                        out=ps[:, 0, 0:n], lhsT=wk[g], rhs=xb[:, j0 : j0 + n],
                        start=True, stop=True,
                    )
                    drain_copy(kt[:, j0 : j0 + n], ps[:, 0, 0:n])
                kg[g] = kt

            qg[0] = pp.tile([128, NSL], BF16, tag="qg0", name="qg0")
            qg[1] = pp.tile([128, NSL], BF16, tag="qg1", name="qg1")

            def emit_q_block(g, blk, ps):
                xqv = xq.rearrange("p (r c) -> p r c", c=50)
                for ty in range(3):
                    for tx in range(3):
                        tap = 3 * ty + tx
                        nc.tensor.matmul(
                            out=ps[:, 0, 0:384],
                            lhsT=w2[9 * g + tap],
                            rhs=xqv[:, 8 * blk + ty : 8 * blk + ty + 8,
                                    tx : tx + W],
                            start=(tap == 0), stop=(tap == 8),
                        )
                nc.vector.tensor_scalar_add(
                    out=qg[g][:, blk * 384 : (blk + 1) * 384],
                    in0=ps[:, 0, 0:384],
                    scalar1=bqv[g],
                )

            vt = pp.tile([128, NI, 136], BF16, tag="vt")

            def emit_vt(i):
                # bank 7 halves; free during the first j-tile's units
                col = 256 * (i % 2)
                nc.tensor.matmul(
                    out=lpbig[:, 7, col : col + 136],
                    lhsT=xb[:, i * MT : (i + 1) * MT],
                    rhs=wv, start=True, stop=True,
                )
                if i % 2 == 1:
                    pair = lpbig[:, 7, :].rearrange(
                        "p (a b) -> p a b", b=256)[:, :, 0:136]
                    nc.vector.tensor_add(
                        out=vt[:, i - 1 : i + 1, :], in0=pair,
                        in1=bvr.rearrange("p (a b) -> p a b", a=1
                                          ).broadcast_to([128, 2, 136]),
                    )

            emit_k(0)
            emit_k(1)
            emit_q_block(0, 0, psum_bank())
            emit_q_block(1, 0, psum_bank())

            # ---- attention main loop ----
            o2 = pp.tile([DIM, NSL], BF16, tag="o2")
            sqd = pp.tile([DIM, NSL], BF16, tag="sqd")
            s1p = pp.tile([DIM, NJ], F32, tag="s1p")
            s2p = pp.tile([DIM, NJ], F32, tag="s2p")
            units = [(j, i) for j in range(NJ) for i in range(NI)]

            def emit_logits(u):
                j, i = units[u]
                z = u % 3
                js = slice(j * NT, (j + 1) * NT)
                for g in range(2):
                    for jj in range(4):
                        h = 4 * g + jj
                        nc.tensor.matmul(
                            out=lpbig[:, 2 * z + h // 4,
                                      128 * (h % 4) : 128 * (h % 4) + 128],
                            lhsT=kg[g][32 * jj : 32 * jj + 16,
                                       i * MT : (i + 1) * MT],
                            rhs=qg[g][32 * jj : 32 * jj + 16, js],
                            start=True, stop=True,
                            tile_position=(32 * jj, 0),
                        )

            def emit_exp(u):
                z = u % 3
                src = lpbig[:, 2 * z : 2 * z + 2, :].rearrange("p a b -> p (a b)")
                pt = ptp.tile([128, 8, 128], BF16, tag="pt", name="pt")
                flat = pt.rearrange("p a b -> p (a b)")
                if (u % EXP_DVE_MOD) in EXP_DVE_PAT:
                    nc.vector._custom_dve(
                        EXP4, out=flat, in0=src,
                        in1=k1t[:, 0:1].broadcast_to([128, 1024]),
                        s0=K4, s1=K3, imm2=K2,
                    )
                else:
                    nc.scalar.activation(out=flat, in_=src, func=AF.Exp, scale=0.25)
                return pt

            def emit_av(u, pt):
                j, i = units[u]
                half = 256 * (j % 2)  # double-buffered AV accumulator
                for h in range(HEADS):
                    nc.tensor.matmul(
                        out=lpbig[:, 6, half + 32 * h : half + 32 * h + 17],
                        lhsT=pt[:, h, :],
                        rhs=vt[:, i, 17 * h : 17 * h + 17],
                        start=(i == 0), stop=(i == NI - 1),
                    )

            def emit_finalize(j):
                js = slice(j * NT, (j + 1) * NT)
                # 1/S for the 8 heads (S at col 32h of the AV bank)
                half = 256 * (j % 2)
                acc = lpbig[:, 6, half : half + 256].rearrange(
                    "p (a b) -> p a b", b=32)
                rrec = finp.tile([128, 8, 1], F32, tag="rrec", name="rrec")
                nc.vector.reciprocal(out=rrec, in_=acc[:, :, 0:1])
                # attT[n, 17h+t] = relu(acc[n, 32h+t]) * rrec[n, h]
                attT = finp.tile([128, 8, 17], BF16, tag="attT", name="attT")
                nc.vector.scalar_tensor_tensor(
                    out=attT, in0=acc[:, :, 0:17], scalar=0.0,
                    in1=rrec.broadcast_to([128, 8, 17]),
                    op0=ALU.max, op1=ALU.mult,
                )
                # transpose to channel-major: two [128,68] -> [68,128] into bank7
                attF = attT.rearrange("p a b -> p (a b)")
                nc.tensor.transpose(
                    out=lpb16[0:68, 7, 0:128], in_=attF[:, 0:68], identity=idn)
                nc.tensor.transpose(
                    out=lpb16[0:68, 7, 128:256], in_=attF[:, 68:136], identity=idn)
                projin = finp.tile([68, 2, 128], BF16, tag="projin", name="projin")
                drain_copy(projin,
                           lpb16[0:68, 7, 0:256].rearrange("p (a b) -> p a b", a=2))
                # 1x1 proj (no bias): contraction over the 136 channels in 2 chunks
                for c in range(2):
                    nc.tensor.matmul(
                        out=lpbig[:, 7, 128:256],
                        lhsT=wpj[:, c, :], rhs=projin[:, c, :],
                        start=(c == 0), stop=(c == 1),
                    )
                drain_copy(o2[:, js], lpbig[:, 7, 128:256])
                nc.vector.tensor_reduce(
                    out=s1p[:, j : j + 1], in_=o2[:, js],
                    op=ALU.add, axis=mybir.AxisListType.X,
                )
                nc.vector.scalar_tensor_tensor(
                    out=sqd[:, js], in0=o2[:, js], scalar=1.0, in1=o2[:, js],
                    op0=ALU.mult, op1=ALU.mult, accum_out=s2p[:, j : j + 1],
                )

            NU = len(units)
            pt_hold = deque()
            qlate = [(0, 1), (1, 1), (0, 2), (1, 2)]
            for u in range(NU + 2):
                if u < NI:
                    emit_vt(u)
                elif NI <= u < NI + 4:
                    g_, b_ = qlate[u - NI]
                    emit_q_block(g_, b_, lpbig[:, 7:8, :])
                if u < NU:
                    emit_logits(u)
                if 0 <= u - 1 < NU:
                    pt_hold.append(emit_exp(u - 1))
                if 0 <= u - 2 < NU:
                    emit_av(u - 2, pt_hold.popleft())
                    jj_, ii_ = units[u - 2]
                    if ii_ == NI - 1:
                        emit_finalize(jj_)

            # ---- GroupNorm ----
            s12 = pp.tile([DIM, 2], F32, tag="s12")
            nc.vector.tensor_reduce(
                out=s12[:, 0:1], in_=s1p, op=ALU.add, axis=mybir.AxisListType.X
            )
            nc.vector.tensor_reduce(
                out=s12[:, 1:2], in_=s2p, op=ALU.add, axis=mybir.AxisListType.X
            )
            s12r = pp.tile([DIM, 2], F32R, tag="s12r")
            nc.vector.tensor_copy(out=s12r, in_=s12)
            gselr = pp.tile([DIM, 8], F32R, tag="gselr")
            nc.vector.tensor_copy(out=gselr, in_=gsel)
            gp = psum_bank()
            nc.tensor.matmul(
                out=gp[0:8, 0, 0:2], lhsT=gselr, rhs=s12r, start=True, stop=True
            )
            gst = pp.tile([8, 2], F32, tag="gst")
            nc.vector.tensor_copy(out=gst, in_=gp[0:8, 0, 0:2])
            ccw = nc.gpsimd.dma_start(out=cc_in[:, :], in_=gst)
            if with_cc:
                cci = nc.gpsimd.collective_compute(
                    "AllReduce", ALU.add,
                    ins=[cc_in[:, :]], outs=[cc_out[:, :]],
                    replica_groups=[[0, 1], [2, 3], [4, 5], [6, 7]],
                )
            else:
                cci = nc.gpsimd.dma_start(out=cc_out[:, :], in_=cc_in[:, :])
            add_dep_helper(cci.ins, ccw.ins, reason="cc_in RAW")
            gch = pp.tile([DIM, 2], F32, tag="gch")
            ccr = nc.gpsimd.dma_start(
                out=gch,
                in_=bass.AP(
                    tensor=cc_out[:, :].tensor, offset=0,
                    ap=[[2, 8], [0, 16], [1, 2]],
                ),
            )
            add_dep_helper(ccr.ins, cci.ins, reason="cc_out RAW")
            # mu, var -> rstd = exp(-0.5*ln(var+eps)); A = rstd*gamma;
            # Bc = beta - mu*A; out = o2*A + Bc
            mu = pp.tile([DIM, 1], F32, tag="mu")
            nc.vector.tensor_scalar_mul(out=mu, in0=gch[:, 0:1], scalar1=GN_DIV)
            ex2 = pp.tile([DIM, 1], F32, tag="ex2")
            nc.vector.tensor_scalar_mul(out=ex2, in0=gch[:, 1:2], scalar1=GN_DIV)
            mu2 = pp.tile([DIM, 1], F32, tag="mu2")
            nc.vector.tensor_mul(out=mu2, in0=mu, in1=mu)
            var = pp.tile([DIM, 1], F32, tag="var")
            nc.vector.tensor_sub(out=var, in0=ex2, in1=mu2)
            epst = pp.tile([DIM, 1], F32, tag="epst")
            nc.vector.memset(epst, EPS)
            lnv = pp.tile([DIM, 1], F32, tag="lnv")
            nc.scalar.activation(out=lnv, in_=var, func=AF.Ln, bias=epst)
            rstd = pp.tile([DIM, 1], F32, tag="rstd")
            nc.scalar.activation(out=rstd, in_=lnv, func=AF.Exp, scale=-0.5)
            A = pp.tile([DIM, 1], F32, tag="A")
            nc.vector.tensor_mul(out=A, in0=rstd, in1=gab[:, 0:1])
            muA = pp.tile([DIM, 1], F32, tag="muA")
            nc.vector.tensor_mul(out=muA, in0=mu, in1=A)
            Bc = pp.tile([DIM, 1], F32, tag="Bc")
            nc.vector.tensor_sub(out=Bc, in0=gab[:, 1:2], in1=muA)
            of = pp.tile([DIM, NSL], F32, tag="of")
            nc.gpsimd.tensor_scalar(
                out=of, in0=o2, scalar1=A, scalar2=Bc,
                op0=ALU.mult, op1=ALU.add,
            )
            nc.sync.dma_start(out=out_d[:, :], in_=of)

    _split_multi_waits(nc)
    return nc


_CACHE = {}


def _prep(w_qkv, b_qkv, w_dw, b_dw, w_proj, gn_w, gn_b):
    """Host-side weight layout prep. head h = 4g+jj, channel ch = 16h+d."""
    ch = lambda h, d: 16 * h + d
    wk = np.zeros((2, DIM, 128), np.float32)
    w2 = np.zeros((2, 9, DIM, 128), np.float32)
    bq = np.zeros((2, 128), np.float32)
    wv = np.zeros((DIM, 136), np.float32)
    bvr = np.zeros((128, 136), np.float32)
    wpj = np.zeros((68, 2, 128), np.float32)
    dwsum = w_dw[:, 0].sum(axis=(1, 2))  # [128]
    for g in range(2):
        for jj in range(4):
            h = 4 * g + jj
            for d in range(16):
                c = ch(h, d)
                p = 32 * jj + d
                wk[g, :, p] = w_qkv[128 + c, :]
                bq[g, p] = b_qkv[c] * dwsum[c] + b_dw[c]
                for tap in range(9):
                    ty, tx = tap // 3, tap % 3
                    w2[g, tap, :, p] = w_dw[c, 0, ty, tx] * w_qkv[c, :]
    for h in range(HEADS):
        bvr[:, 17 * h] = 1.0
        for d in range(16):
            c = ch(h, d)
            wv[:, 17 * h + 1 + d] = w_qkv[256 + c, :]
            bvr[:, 17 * h + 1 + d] = b_qkv[256 + c]
            cp = 17 * h + 1 + d   # attT channel index
            wpj[cp % 68, cp // 68, :] = w_proj[:, c]
    gab = np.stack([gn_w, gn_b], axis=1).astype(np.float32)
    gsel = np.zeros((DIM, 8), np.float32)
    for c in range(DIM):
        gsel[c, c // 16] = 1.0
    idn = np.eye(DIM, 128, dtype=np.float32)
    # pad pixel x-vector: projects exactly to -b_q so biased q is 0 there
    vpad = -np.linalg.solve(w_qkv[0:128, :].astype(np.float64),
                            b_qkv[0:128].astype(np.float64)).astype(np.float32)
    weights = dict(
        wk=wk.transpose(1, 0, 2).astype(NPBF16),
        w2=w2.reshape(18, DIM, 128).transpose(1, 0, 2).astype(NPBF16),
        bq=bq.T.copy(),
        wv=wv.astype(NPBF16), bvr=bvr.astype(NPBF16),
        wpj=wpj.astype(NPBF16), idn=idn.astype(NPBF16),
        gab=gab, gsel=gsel,
    )
    return weights, vpad


def kernel(x, w_qkv, b_qkv, w_dw, b_dw, w_proj, gn_w, gn_b):
    x = np.asarray(x, np.float32)
    w_qkv = np.asarray(w_qkv, np.float32)
    b_qkv = np.asarray(b_qkv, np.float32)
    w_dw = np.asarray(w_dw, np.float32)
    b_dw = np.asarray(b_dw, np.float32)
    w_proj = np.asarray(w_proj, np.float32)
    gn_w = np.asarray(gn_w, np.float32)
    gn_b = np.asarray(gn_b, np.float32)

    weights, vpad = _prep(w_qkv, b_qkv, w_dw, b_dw, w_proj, gn_w, gn_b)

    if "nc" not in _CACHE:
        _CACHE["nc"] = _build()
    nc = _CACHE["nc"]

    in_maps = []
    for c in range(8):
        b, s = c // 2, c % 2
        xb = x[b].reshape(DIM, N)
        # q source: image rows 24s-1 .. 24s+24 with vpad padding (rows and
        # cols) so the post-projection-biased q is exactly 0 on the halo
        xq = np.empty((DIM, 26, 50), np.float32)
        xq[:, :, :] = vpad[:, None, None]
        xv = x[b]  # [DIM, H, W]
        if s == 0:
            xq[:, 1:26, 1:49] = xv[:, 0:25, :]
        else:
            xq[:, 0:25, 1:49] = xv[:, 23:48, :]
        m = {"xb": np.ascontiguousarray(xb).astype(NPBF16),
             "xq": xq.reshape(DIM, 26 * 50).astype(NPBF16)}
        m.update(weights)
        in_maps.append(m)

    res = run_bass_kernel_spmd(nc, in_maps, core_ids=list(range(8)))

    out = np.empty((B, DIM, H, W), np.float32)
    for c in range(8):
        b, s = c // 2, c % 2
        out[b, :, 24 * s : 24 * s + 24, :] = res.results[c]["out_half"].reshape(
            DIM, ROWS_HALF, W
        )
    return out
